# revision 1
# baseline (speedup 1.0000x reference)
"""HINGCN edge-emb GNN message passing on 8 Trainium2 NeuronCores.

Strategy: data-parallel over the queried-vertex batch B. Each core gets
B/8 queries (padded to a multiple of 128); node_emb / edge tables /
weights are replicated. All gathers (edge_index rows, edge_emb rows,
neighbor node features) run on-device via indirect DMA.

Per 128-query tile and metapath:
  nbrs  = edge_index[m][idx]            (indirect DMA, 128B rows)
  erows = edge_emb[m]  [idx]            (indirect DMA, 4KB rows)
  G     = node_emb[nbrs]                (indirect DMA, 512B rows)
  k-scores / e-scores: DVE mult + strided reduce
  softmax (DVE/ACT), attention-weighted sum (scalar_tensor_tensor chain)
  agg = wsum @ Wk via PE (transpose + matmul), elu, second layer, then
  metapath-attention fusion + classifier + log_softmax.
"""

import math
import sys

for _p in ("/opt/trn_rl_repo",):
    if _p not in sys.path:
        sys.path.insert(0, _p)

import numpy as np

import concourse.bacc as bacc
import concourse.bass as bass
import concourse.mybir as mybir
from concourse.bass import IndirectOffsetOnAxis
from concourse.masks import make_identity
from concourse.tile import TileContext

F32 = mybir.dt.float32
I32 = mybir.dt.int32
AX = mybir.AxisListType
OP = mybir.AluOpType
ACT = mybir.ActivationFunctionType

NCORES = 8
T = 128          # queries per tile (partition dim)
NB = 32          # neighbor table width
NFEAT = 128
NHID = 64
DIM_MP = 64
EDIM = 32
NMETA = 3
NCLASS = 8
ALPHA = 0.2


def build_nc(n_nodes: int, nt: int, S: int, dbg: bool = False):
    """Build the single-core Bass program. nt = tiles of 128 queries."""
    nc = bacc.Bacc("TRN2", target_bir_lowering=False, debug=False)
    b_core = nt * T

    inp = nc.dram_tensor("inp", [b_core, NFEAT], F32, kind="ExternalInput").ap()
    idxd = nc.dram_tensor("idxd", [T, nt], I32, kind="ExternalInput").ap()
    nemb = nc.dram_tensor("nemb", [n_nodes, NFEAT], F32, kind="ExternalInput").ap()
    eid = [
        nc.dram_tensor(f"ei{m}", [n_nodes, NB], I32, kind="ExternalInput").ap()
        for m in range(NMETA)
    ]
    eed = [
        nc.dram_tensor(f"ee{m}", [n_nodes, NB * EDIM], F32, kind="ExternalInput").ap()
        for m in range(NMETA)
    ]
    wq1d = nc.dram_tensor("wq1", [NMETA, NFEAT, NHID], F32, kind="ExternalInput").ap()
    wk1d = nc.dram_tensor("wk1", [NMETA, NFEAT, NHID], F32, kind="ExternalInput").ap()
    a1d = nc.dram_tensor("a1", [NMETA, 2 * NHID + EDIM], F32, kind="ExternalInput").ap()
    wq2d = nc.dram_tensor("wq2", [NMETA, NHID, DIM_MP], F32, kind="ExternalInput").ap()
    wk2d = nc.dram_tensor("wk2", [NMETA, NFEAT, DIM_MP], F32, kind="ExternalInput").ap()
    a2d = nc.dram_tensor("a2", [NMETA, 2 * DIM_MP + EDIM], F32, kind="ExternalInput").ap()
    ampd = nc.dram_tensor("amp", [DIM_MP], F32, kind="ExternalInput").ap()
    wcd = nc.dram_tensor("wc", [DIM_MP, NCLASS], F32, kind="ExternalInput").ap()
    bcd = nc.dram_tensor("bc", [NCLASS], F32, kind="ExternalInput").ap()
    outd = nc.dram_tensor("outp", [b_core, NCLASS], F32, kind="ExternalOutput").ap()
    if dbg:
        dbgd = {
            "dbg_G": nc.dram_tensor("dbg_G", [T, S * NFEAT], F32, kind="ExternalOutput").ap(),
            "dbg_k1": nc.dram_tensor("dbg_k1", [T, S], F32, kind="ExternalOutput").ap(),
            "dbg_e1": nc.dram_tensor("dbg_e1", [T, S], F32, kind="ExternalOutput").ap(),
            "dbg_att1": nc.dram_tensor("dbg_att1", [T, S], F32, kind="ExternalOutput").ap(),
            "dbg_ws1": nc.dram_tensor("dbg_ws1", [T, NFEAT], F32, kind="ExternalOutput").ap(),
            "dbg_x1": nc.dram_tensor("dbg_x1", [T, NHID], F32, kind="ExternalOutput").ap(),
            "dbg_q1": nc.dram_tensor("dbg_q1", [T, NMETA], F32, kind="ExternalOutput").ap(),
            "dbg_u1": nc.dram_tensor("dbg_u1", [128, NFEAT], F32, kind="ExternalOutput").ap(),
        }

    with TileContext(nc) as tc:
        with (
            tc.tile_pool(name="persist", bufs=1) as pp,
            tc.tile_pool(name="prep", bufs=2) as prep,
            tc.tile_pool(name="gpool", bufs=2) as gpool,
            tc.tile_pool(name="spool", bufs=2) as spool,
            tc.tile_pool(name="small", bufs=3) as sm,
            tc.tile_pool(name="psum", bufs=2, space="PSUM") as ps,
        ):
            # ---------------- preamble: identity, weights, derived vecs
            ident = pp.tile([128, 128], F32, name="ident")
            make_identity(nc, ident[:])
            ones1 = pp.tile([1, 128], F32, name="ones1")
            nc.vector.memset(ones1[:], 1.0)

            def brow(row, width, name):
                """Replicate a [1,width] row across 128 partitions."""
                p = ps.tile([128, width], F32, tag="prep_ps", name=f"{name}_bp")
                nc.tensor.matmul(out=p[:], lhsT=ones1[:], rhs=row[0:1, :])
                t = pp.tile([128, width], F32, name=name)
                nc.vector.tensor_copy(out=t[:], in_=p[:])
                return t

            idxs = pp.tile([T, nt], I32, name="idxs")
            nc.sync.dma_start(out=idxs[:], in_=idxd[:, :])

            # per-metapath persistent weights / vectors
            U1 = []   # [1,128] u1 row (Wk1 @ a1_mid)
            U2 = []   # [1,128]
            V2 = []   # [1,64] v2 row (Wq2 @ a2_lo)
            AE1 = []  # [1,32]
            AE2 = []  # [1,32]
            WK1 = []  # [128,64]
            WK2 = []  # [128,64]
            V1cols = pp.tile([NFEAT, NMETA], F32, name="V1cols")

            for m in range(NMETA):
                wk1_m = pp.tile([NFEAT, NHID], F32, name=f"wk1_{m}")
                nc.sync.dma_start(out=wk1_m[:], in_=wk1d[m])
                wk2_m = pp.tile([NFEAT, DIM_MP], F32, name=f"wk2_{m}")
                nc.sync.dma_start(out=wk2_m[:], in_=wk2d[m])
                WK1.append(wk1_m)
                WK2.append(wk2_m)

                wq1_m = prep.tile([NFEAT, NHID], F32, tag="wq_m")
                nc.sync.dma_start(out=wq1_m[:], in_=wq1d[m])
                wq2_m = prep.tile([NHID, DIM_MP], F32, tag="wq2_m")
                nc.sync.dma_start(out=wq2_m[:], in_=wq2d[m])

                a1lo = prep.tile([NHID, 1], F32, tag="alo")
                nc.sync.dma_start(out=a1lo[:], in_=a1d[m, 0:NHID, None])
                a1mid = prep.tile([NHID, 1], F32, tag="amid")
                nc.sync.dma_start(out=a1mid[:], in_=a1d[m, NHID : 2 * NHID, None])
                a2lo = prep.tile([DIM_MP, 1], F32, tag="a2lo")
                nc.sync.dma_start(out=a2lo[:], in_=a2d[m, 0:DIM_MP, None])
                a2mid = prep.tile([DIM_MP, 1], F32, tag="a2mid")
                nc.sync.dma_start(out=a2mid[:], in_=a2d[m, DIM_MP : 2 * DIM_MP, None])

                ae1r = prep.tile([1, EDIM], F32, tag="ae1r")
                nc.sync.dma_start(out=ae1r[:], in_=a1d[m, None, 2 * NHID :])
                ae2r = prep.tile([1, EDIM], F32, tag="ae2r")
                nc.sync.dma_start(out=ae2r[:], in_=a2d[m, None, 2 * DIM_MP :])
                AE1.append(brow(ae1r, EDIM, f"ae1b_{m}"))
                AE2.append(brow(ae2r, EDIM, f"ae2b_{m}"))

                # transposes of Wk1/Wq1/Wk2/Wq2
                def _tp(dst_shape, src, tag):
                    kk = src.shape[0]
                    p = ps.tile([dst_shape[0], dst_shape[1]], F32, tag="prep_ps", name="tp_ps")
                    nc.tensor.transpose(
                        out=p[:], in_=src[:], identity=ident[0:kk, 0:kk]
                    )
                    t = prep.tile(dst_shape, F32, tag=tag)
                    nc.vector.tensor_copy(out=t[:], in_=p[:])
                    return t

                wk1t = _tp([NHID, NFEAT], wk1_m, "wk1t")
                wq1t = _tp([NHID, NFEAT], wq1_m, "wq1t")
                wk2t = _tp([DIM_MP, NFEAT], wk2_m, "wk2t")
                wq2t = _tp([DIM_MP, NHID], wq2_m, "wq2t")

                # u rows: [1, 128] = (a_mid^T @ WkT)
                u1p = ps.tile([1, NFEAT], F32, tag="prep_ps", name="urow_ps")
                nc.tensor.matmul(out=u1p[:], lhsT=a1mid[:], rhs=wk1t[:])
                u1 = prep.tile([1, NFEAT], F32, tag="u1row")
                nc.vector.tensor_copy(out=u1[:], in_=u1p[:])
                u1 = brow(u1, NFEAT, f"u1b_{m}")
                u2p = ps.tile([1, NFEAT], F32, tag="prep_ps", name="urow_ps")
                nc.tensor.matmul(out=u2p[:], lhsT=a2mid[:], rhs=wk2t[:])
                u2 = prep.tile([1, NFEAT], F32, tag="u2row")
                nc.vector.tensor_copy(out=u2[:], in_=u2p[:])
                u2 = brow(u2, NFEAT, f"u2b_{m}")
                U1.append(u1)
                U2.append(u2)

                # v1 column: [128, 1] = Wq1 @ a1_lo   (lhsT = Wq1T)
                v1p = ps.tile([NFEAT, 1], F32, tag="prep_ps", name="vcol_ps")
                nc.tensor.matmul(out=v1p[:], lhsT=wq1t[:], rhs=a1lo[:])
                nc.vector.tensor_copy(out=V1cols[:, m : m + 1], in_=v1p[:])

                # v2 row: [1, 64] = a2_lo^T @ Wq2T
                v2p = ps.tile([1, NHID], F32, tag="prep_ps", name="v2_ps")
                nc.tensor.matmul(out=v2p[:], lhsT=a2lo[:], rhs=wq2t[:])
                v2 = prep.tile([1, NHID], F32, tag="v2row")
                nc.vector.tensor_copy(out=v2[:], in_=v2p[:])
                v2 = brow(v2, NHID, f"v2b_{m}")
                V2.append(v2)

            ampr = prep.tile([1, DIM_MP], F32, tag="ampr")
            nc.sync.dma_start(out=ampr[:], in_=ampd[None, :])
            amp = brow(ampr, DIM_MP, "ampb")
            wc = pp.tile([DIM_MP, NCLASS], F32, name="wc")
            nc.sync.dma_start(out=wc[:], in_=wcd[:, :])
            bcr0 = prep.tile([1, NCLASS], F32, tag="bcr0")
            nc.sync.dma_start(out=bcr0[:], in_=bcd[None, :])
            bcr = brow(bcr0, NCLASS, "bcb")

            # inputT [128, b_core] and Q1 [128, nt*3]
            inputT = pp.tile([NFEAT, b_core], F32, name="inputT")
            Q1 = pp.tile([T, nt * NMETA], F32, name="Q1")
            for t in range(nt):
                itile = prep.tile([T, NFEAT], F32, tag="itile")
                nc.sync.dma_start(out=itile[:], in_=inp[t * T : (t + 1) * T, :])
                itp = ps.tile([NFEAT, T], F32, tag="prep_ps", name="itp_ps")
                nc.tensor.transpose(out=itp[:], in_=itile[:], identity=ident[:])
                nc.vector.tensor_copy(out=inputT[:, t * T : (t + 1) * T], in_=itp[:])
                q1p = ps.tile([T, NMETA], F32, tag="prep_ps", name="q1_ps")
                nc.tensor.matmul(
                    out=q1p[:], lhsT=inputT[:, t * T : (t + 1) * T], rhs=V1cols[:]
                )
                nc.vector.tensor_copy(
                    out=Q1[:, t * NMETA : (t + 1) * NMETA], in_=q1p[:]
                )

            OUTS = pp.tile([T, nt * NCLASS], F32, name="OUTS")

            # ---------------- helpers
            def softmax_att(scores, qcol):
                """scores [T,S] (+ qcol [T,1] bias) -> att [T,S]."""
                sq = sm.tile([T, S], F32, tag="sq")
                nc.vector.tensor_scalar_add(out=sq[:], in0=scores[:], scalar1=qcol)
                sl = sm.tile([T, S], F32, tag="sl")
                nc.vector.scalar_tensor_tensor(
                    out=sl[:], in0=sq[:], scalar=ALPHA, in1=sq[:],
                    op0=OP.mult, op1=OP.max,
                )
                ex = sm.tile([T, S], F32, tag="ex")
                nc.scalar.activation(out=ex[:], in_=sl[:], func=ACT.Exp)
                ssum = sm.tile([T, 1], F32, tag="ssum")
                nc.vector.reduce_sum(out=ssum[:], in_=ex[:], axis=AX.X)
                rec = sm.tile([T, 1], F32, tag="rec")
                nc.vector.reciprocal(out=rec[:], in_=ssum[:])
                att = sm.tile([T, S], F32, tag="att")
                nc.vector.tensor_scalar_mul(out=att[:], in0=ex[:], scalar1=rec[:, 0:1])
                return att

            def weighted_sum(G, att, width):
                """wsum[p,f] = sum_s att[p,s] * G[p, s*width:(s+1)*width]."""
                acc = [
                    sm.tile([T, width], F32, tag="acc0", name="acc0"),
                    sm.tile([T, width], F32, tag="acc1", name="acc1"),
                ]
                nc.vector.tensor_scalar_mul(
                    out=acc[0][:], in0=G[:, 0:width], scalar1=att[:, 0:1]
                )
                for s in range(1, S):
                    src = acc[(s + 1) % 2]
                    dst = acc[s % 2]
                    nc.vector.scalar_tensor_tensor(
                        out=dst[:],
                        in0=G[:, s * width : (s + 1) * width],
                        scalar=att[:, s : s + 1],
                        in1=src[:],
                        op0=OP.mult,
                        op1=OP.add,
                    )
                return acc[(S - 1) % 2]

            def elu(ag_psum, width):
                rl = sm.tile([T, width], F32, tag="elu_rl")
                nc.vector.tensor_scalar_max(out=rl[:], in0=ag_psum[:], scalar1=0.0)
                mn = sm.tile([T, width], F32, tag="elu_mn")
                nc.vector.tensor_scalar_min(out=mn[:], in0=ag_psum[:], scalar1=0.0)
                exm = sm.tile([T, width], F32, tag="elu_ex")
                nc.scalar.activation(out=exm[:], in_=mn[:], func=ACT.Exp)
                x = sm.tile([T, width], F32, tag="elu_x")
                nc.vector.scalar_tensor_tensor(
                    out=x[:], in0=exm[:], scalar=-1.0, in1=rl[:], op0=OP.add, op1=OP.add
                )
                return x

            def dot_rows(x, vrow, width, tag):
                """[T,1] = sum_f x[p,f] * vrow[0,f]."""
                mv = sm.tile([T, width], F32, tag=f"{tag}_mv")
                nc.vector.tensor_tensor(
                    out=mv[:], in0=x[:], in1=vrow[:, :], op=OP.mult
                )
                r = sm.tile([T, 1], F32, tag=f"{tag}_r")
                nc.vector.reduce_sum(out=r[:], in_=mv[:], axis=AX.X)
                return r

            def kscore(G, urow):
                """[T,S] = sum_f G[p,s,f] * urow[0,f]."""
                mk = spool.tile([T, S * NFEAT], F32, tag="mk")
                nc.vector.tensor_tensor(
                    out=mk[:],
                    in0=G[:],
                    in1=urow[:, None, :].to_broadcast([T, S, NFEAT]),
                    op=OP.mult,
                )
                r = sm.tile([T, S], F32, tag="ks")
                nc.vector.reduce_sum(
                    out=r[:],
                    in_=mk[:].rearrange("p (s f) -> p s f", f=NFEAT),
                    axis=AX.X,
                )
                return r

            def escore(erows, aerow):
                me = spool.tile([T, S * EDIM], F32, tag="me")
                nc.vector.tensor_tensor(
                    out=me[:],
                    in0=erows[:, 0 : S * EDIM],
                    in1=aerow[:, None, :].to_broadcast([T, S, EDIM]),
                    op=OP.mult,
                )
                r = sm.tile([T, S], F32, tag="es")
                nc.vector.reduce_sum(
                    out=r[:],
                    in_=me[:].rearrange("p (s e) -> p s e", e=EDIM),
                    axis=AX.X,
                )
                return r

            def project(wsum, wk, width_out, tag):
                """agg[T, width_out] = wsum[T,128] @ wk[128, width_out] (PSUM)."""
                wtp = ps.tile([NFEAT, T], F32, tag="wtp", name="wtp", bufs=3)
                nc.tensor.transpose(out=wtp[:], in_=wsum[:], identity=ident[:])
                wts = sm.tile([NFEAT, T], F32, tag=f"{tag}_wts")
                nc.vector.tensor_copy(out=wts[:], in_=wtp[:])
                ag = ps.tile([T, width_out], F32, tag="ag", name="ag", bufs=3)
                nc.tensor.matmul(out=ag[:], lhsT=wts[:], rhs=wk[:])
                return ag

            # ---------------- main loop
            for t in range(nt):
                x2s = sm.tile([T, NMETA * DIM_MP], F32, tag="x2s")
                for m in range(NMETA):
                    nbrs = gpool.tile([T, NB], I32, tag="nbrs")
                    nc.gpsimd.indirect_dma_start(
                        out=nbrs[:],
                        out_offset=None,
                        in_=eid[m][:, :],
                        in_offset=IndirectOffsetOnAxis(ap=idxs[:, t : t + 1], axis=0),
                    )
                    erows = gpool.tile([T, NB * EDIM], F32, tag="erows")
                    nc.gpsimd.indirect_dma_start(
                        out=erows[:],
                        out_offset=None,
                        in_=eed[m][:, :],
                        in_offset=IndirectOffsetOnAxis(ap=idxs[:, t : t + 1], axis=0),
                    )
                    G = gpool.tile([T, S * NFEAT], F32, tag="G")
                    # one gather per sampled neighbor column: multi-index
                    # offset APs land in a different layout on HW than in sim
                    for s_ in range(S):
                        nc.gpsimd.indirect_dma_start(
                            out=G[:, s_ * NFEAT : (s_ + 1) * NFEAT],
                            out_offset=None,
                            in_=nemb[:, :],
                            in_offset=IndirectOffsetOnAxis(
                                ap=nbrs[:, s_ : s_ + 1], axis=0
                            ),
                        )

                    # ---- layer 1
                    k1 = kscore(G, U1[m])
                    if dbg and t == 0 and m == 0:
                        nc.sync.dma_start(out=dbgd["dbg_G"][:, :], in_=G[:])
                        nc.sync.dma_start(out=dbgd["dbg_k1"][:, :], in_=k1[:])
                        nc.sync.dma_start(out=dbgd["dbg_u1"][:, :], in_=U1[0][:])
                        nc.sync.dma_start(out=dbgd["dbg_q1"][:, :], in_=Q1[:, 0:NMETA])
                    e1 = escore(erows, AE1[m])
                    st1 = sm.tile([T, S], F32, tag="st")
                    nc.vector.tensor_add(out=st1[:], in0=k1[:], in1=e1[:])
                    att1 = softmax_att(st1, Q1[:, t * NMETA + m : t * NMETA + m + 1])
                    ws1 = weighted_sum(G, att1, NFEAT)
                    ag1 = project(ws1, WK1[m], NHID, "l1")
                    x1 = elu(ag1, NHID)
                    if dbg and t == 0 and m == 0:
                        nc.sync.dma_start(out=dbgd["dbg_e1"][:, :], in_=e1[:])
                        nc.sync.dma_start(out=dbgd["dbg_att1"][:, :], in_=att1[:])
                        nc.sync.dma_start(out=dbgd["dbg_ws1"][:, :], in_=ws1[:])
                        nc.sync.dma_start(out=dbgd["dbg_x1"][:, :], in_=x1[:])

                    # ---- layer 2
                    q2 = dot_rows(x1, V2[m], NHID, "q2")
                    k2 = kscore(G, U2[m])
                    e2 = escore(erows, AE2[m])
                    st2 = sm.tile([T, S], F32, tag="st")
                    nc.vector.tensor_add(out=st2[:], in0=k2[:], in1=e2[:])
                    att2 = softmax_att(st2, q2[:, 0:1])
                    ws2 = weighted_sum(G, att2, NFEAT)
                    ag2 = project(ws2, WK2[m], DIM_MP, "l2")
                    x2 = elu(ag2, DIM_MP)
                    nc.vector.tensor_copy(
                        out=x2s[:, m * DIM_MP : (m + 1) * DIM_MP], in_=x2[:]
                    )

                # ---- metapath fusion
                fsc = sm.tile([T, NMETA], F32, tag="fsc")
                for m in range(NMETA):
                    fm = dot_rows(
                        x2s[:, m * DIM_MP : (m + 1) * DIM_MP], amp, DIM_MP, "fus"
                    )
                    nc.vector.tensor_copy(out=fsc[:, m : m + 1], in_=fm[:])
                fl = sm.tile([T, NMETA], F32, tag="fl")
                nc.vector.scalar_tensor_tensor(
                    out=fl[:], in0=fsc[:], scalar=ALPHA, in1=fsc[:],
                    op0=OP.mult, op1=OP.max,
                )
                fex = sm.tile([T, NMETA], F32, tag="fex")
                nc.scalar.activation(out=fex[:], in_=fl[:], func=ACT.Exp)
                fsum = sm.tile([T, 1], F32, tag="fsum")
                nc.vector.reduce_sum(out=fsum[:], in_=fex[:], axis=AX.X)
                frec = sm.tile([T, 1], F32, tag="frec")
                nc.vector.reciprocal(out=frec[:], in_=fsum[:])
                attm = sm.tile([T, NMETA], F32, tag="attm")
                nc.vector.tensor_scalar_mul(out=attm[:], in0=fex[:], scalar1=frec[:, 0:1])

                fused = [
                    sm.tile([T, DIM_MP], F32, tag="fused0", name="fused0"),
                    sm.tile([T, DIM_MP], F32, tag="fused1", name="fused1"),
                ]
                nc.vector.tensor_scalar_mul(
                    out=fused[0][:], in0=x2s[:, 0:DIM_MP], scalar1=attm[:, 0:1]
                )
                for m in range(1, NMETA):
                    nc.vector.scalar_tensor_tensor(
                        out=fused[m % 2][:],
                        in0=x2s[:, m * DIM_MP : (m + 1) * DIM_MP],
                        scalar=attm[:, m : m + 1],
                        in1=fused[(m + 1) % 2][:],
                        op0=OP.mult,
                        op1=OP.add,
                    )
                fin = fused[(NMETA - 1) % 2]

                # classifier: relu(fused @ Wc + bc)
                ftp = ps.tile([DIM_MP, T], F32, tag="wtp", name="ftp", bufs=3)
                nc.tensor.transpose(out=ftp[:], in_=fin[:], identity=ident[:])
                fts = sm.tile([DIM_MP, T], F32, tag="fts")
                nc.vector.tensor_copy(out=fts[:], in_=ftp[:])
                lg = ps.tile([T, NCLASS], F32, tag="ag", name="lg", bufs=3)
                nc.tensor.matmul(out=lg[:], lhsT=fts[:], rhs=wc[:])
                lb = sm.tile([T, NCLASS], F32, tag="lb")
                nc.vector.tensor_tensor(
                    out=lb[:], in0=lg[:], in1=bcr[:, :], op=OP.add
                )
                lr = sm.tile([T, NCLASS], F32, tag="lr")
                nc.vector.tensor_scalar_max(out=lr[:], in0=lb[:], scalar1=0.0)

                # log_softmax
                mx = sm.tile([T, 1], F32, tag="mx")
                nc.vector.reduce_max(out=mx[:], in_=lr[:], axis=AX.X)
                sh = sm.tile([T, NCLASS], F32, tag="sh")
                nc.vector.tensor_scalar_sub(out=sh[:], in0=lr[:], scalar1=mx[:, 0:1])
                shex = sm.tile([T, NCLASS], F32, tag="shex")
                nc.scalar.activation(out=shex[:], in_=sh[:], func=ACT.Exp)
                sesum = sm.tile([T, 1], F32, tag="sesum")
                nc.vector.reduce_sum(out=sesum[:], in_=shex[:], axis=AX.X)
                lse = sm.tile([T, 1], F32, tag="lse")
                nc.scalar.activation(out=lse[:], in_=sesum[:], func=ACT.Ln)
                nc.vector.tensor_scalar_sub(
                    out=OUTS[:, t * NCLASS : (t + 1) * NCLASS],
                    in0=sh[:],
                    scalar1=lse[:, 0:1],
                )

            nc.sync.dma_start(
                out=outd.rearrange("(t p) c -> p t c", p=T),
                in_=OUTS[:].rearrange("p (t c) -> p t c", c=NCLASS),
            )

    nc.compile()
    return nc


_NC_CACHE: dict = {}
LAST_RESULTS = None


def _get_nc(n_nodes, nt, S):
    key = (n_nodes, nt, S)
    if key not in _NC_CACHE:
        _NC_CACHE[key] = build_nc(n_nodes, nt, S)
    return _NC_CACHE[key]


def kernel(
    input,
    index,
    node_emb,
    edge_index,
    edge_emb,
    n_sample,
    Wq1,
    Wk1,
    a1,
    Wq2,
    Wk2,
    a2,
    a_mp,
    Wc,
    bc,
):
    from concourse.bass_utils import run_bass_kernel_spmd

    input = np.asarray(input, dtype=np.float32)
    index = np.asarray(index)
    node_emb = np.asarray(node_emb, dtype=np.float32)
    edge_index = np.asarray(edge_index, dtype=np.int32)
    edge_emb = np.asarray(edge_emb, dtype=np.float32)
    S = int(n_sample)
    assert 1 <= S <= NB

    B = input.shape[0]
    n_nodes = node_emb.shape[0]
    per = int(math.ceil(B / (NCORES * T))) * T   # queries per core, padded
    nt = per // T
    b_pad = per * NCORES

    inp_p = np.zeros((b_pad, NFEAT), np.float32)
    inp_p[:B] = input
    idx_p = np.zeros((b_pad,), np.int32)
    idx_p[:B] = index.astype(np.int64).astype(np.int32)

    ee3 = edge_emb.reshape(NMETA, n_nodes, NB * EDIM)

    common = {
        "nemb": node_emb,
        "wq1": np.asarray(Wq1, np.float32),
        "wk1": np.asarray(Wk1, np.float32),
        "a1": np.asarray(a1, np.float32),
        "wq2": np.asarray(Wq2, np.float32),
        "wk2": np.asarray(Wk2, np.float32),
        "a2": np.asarray(a2, np.float32),
        "amp": np.asarray(a_mp, np.float32),
        "wc": np.asarray(Wc, np.float32),
        "bc": np.asarray(bc, np.float32),
    }
    for m in range(NMETA):
        common[f"ei{m}"] = np.ascontiguousarray(edge_index[m])
        common[f"ee{m}"] = np.ascontiguousarray(ee3[m])

    in_maps = []
    for c in range(NCORES):
        sl = slice(c * per, (c + 1) * per)
        im = dict(common)
        im["inp"] = np.ascontiguousarray(inp_p[sl])
        im["idxd"] = np.ascontiguousarray(idx_p[sl].reshape(nt, T).T)
        in_maps.append(im)

    nc = _get_nc(n_nodes, nt, S)
    res = run_bass_kernel_spmd(nc, in_maps, core_ids=list(range(NCORES)))
    global LAST_RESULTS
    LAST_RESULTS = res
    out = np.concatenate([res.results[c]["outp"] for c in range(NCORES)], axis=0)
    return out[:B].astype(np.float32)

